# revision 6
# baseline (speedup 1.0000x reference)
"""v7: host-prepped bf16 inputs; x transposed on the DMA xbar (PE-free);
MM2 head-pair row-packing; 578-wide merged exp from 2-bank PSUM tiles;
bias-free MM4; exp(-ln(den)) normalization; DMA queue split (sync=x/weights,
gpsimd=den/bc/y); emission-interleaved schedule with deferred-MM4 tail filler."""
import numpy as np
import concourse.bass as bass
import concourse.mybir as mybir
import concourse.tile as tile

dt = mybir.dt
F32 = dt.float32
BF16 = dt.bfloat16
AF = mybir.ActivationFunctionType

B = 4
T = 577
TPAD = 592          # host zero-pads each batch to 592 tokens (16-aligned for xbar)
D = 768
H = 12
HD = 64
EQK = 1536
SCALE = HD ** -0.5

TT = [(i * 128, min(128, T - i * 128)) for i in range((T + 127) // 128)]
TX = [(i * 128, min(128, TPAD - i * 128)) for i in range(5)]   # xbar transpose tiles
TP = 578
TPX = 592
CH2 = [(0, 512), (512, 66)]     # MM2/MM3 q chunks (bank-aligned, merged ACT)
CH1 = [(0, 512), (512, 66)]     # MM1a token chunks
ECH = [(0, 384), (384, 384)]    # MM1b / MM4 feature chunks
DT = 6


def build(nbatch=B):
    nc = bass.Bass()
    x_d = nc.dram_tensor("x", [B * TPAD, D], BF16, kind="ExternalInput")
    qkwT_d = nc.dram_tensor("qkwT", [D, EQK], BF16, kind="ExternalInput")
    vwT_d = nc.dram_tensor("vwT", [D, D], BF16, kind="ExternalInput")
    pwT_d = nc.dram_tensor("pwT", [D, D], BF16, kind="ExternalInput")
    pb_d = nc.dram_tensor("pb", [1, D], F32, kind="ExternalInput")
    qkb_d = nc.dram_tensor("qkb", [128, 12], F32, kind="ExternalInput")
    y_d = nc.dram_tensor("y", [B * T, D], F32, kind="ExternalOutput")

    from contextlib import ExitStack
    with tile.TileContext(nc) as tc, ExitStack() as ctx:
        wpool = ctx.enter_context(tc.tile_pool(name="wpool", bufs=1))

        xin = ctx.enter_context(tc.tile_pool(name="xin", bufs=2))
        xT_p = ctx.enter_context(tc.tile_pool(name="xT", bufs=2))
        qkT_p = ctx.enter_context(tc.tile_pool(name="qkT", bufs=2))
        v_p = ctx.enter_context(tc.tile_pool(name="v", bufs=2))
        es_p = ctx.enter_context(tc.tile_pool(name="es", bufs=2))
        oT_p = ctx.enter_context(tc.tile_pool(name="oT", bufs=3))
        nrm_p = ctx.enter_context(tc.tile_pool(name="nrm", bufs=2))
        dh_p = ctx.enter_context(tc.tile_pool(name="dh", bufs=4))
        bc_p = ctx.enter_context(tc.tile_pool(name="bc", bufs=3))
        yout = ctx.enter_context(tc.tile_pool(name="yout", bufs=2))
        drp = ctx.enter_context(tc.tile_pool(name="dr", bufs=2, space="DRAM"))

        ps_s = ctx.enter_context(tc.tile_pool(name="ps_s", bufs=1, space="PSUM"))
        ps_o = ctx.enter_context(tc.tile_pool(name="ps_o", bufs=2, space="PSUM"))
        ps_mm = ctx.enter_context(tc.tile_pool(name="ps_mm", bufs=2, space="PSUM"))

        state = {}

        def u_ld(b, ti):
            """x tile load+transpose entirely on the DMA xbar (no PE)"""
            x0 = b * TPAD
            ts_, PP = TX[ti]
            xT = state[b]["xT"]
            for dti in range(DT):
                nc.sync.dma_start_transpose(
                    xT[dti][:, ts_:ts_ + PP],
                    x_d[x0 + ts_: x0 + ts_ + PP, dti * 128:(dti + 1) * 128])

        def u_mm1a(b, et):
            xT = state[b]["xT"]
            qkT = state[b]["qkT"]
            for (cs, cw) in CH1:
                pm = ps_mm.tile([128, 512], F32, tag="mm", name=f"pma_{b}_{et}_{cs}")
                for dti in range(DT):
                    nc.tensor.matmul(pm[:, 0:cw],
                                     qkwT[dti][:, et * 128:(et + 1) * 128],
                                     xT[dti][:, cs:cs + cw],
                                     start=(dti == 0), stop=(dti == DT - 1))
                nc.vector.tensor_scalar_add(qkT[et][:, cs:cs + cw], pm[:, 0:cw],
                                            qkb_sb[:, et:et + 1])

        def u_mm1b(b, ti):
            xT = state[b]["xT"]
            v_sb = state[b]["v"]
            ts_, P = TT[ti]
            vv = v_sb[ti].rearrange("p (h c) -> p h c", c=HD + 1)
            nc.vector.tensor_copy(vv[0:P, :, HD:HD + 1], ones_col[0:P, :].to_broadcast((P, H, 1)))
            for ci, (cs, cw) in enumerate(ECH):
                pm = ps_mm.tile([128, 512], F32, tag="mm", name=f"pmb_{b}_{ti}_{ci}")
                for dti in range(DT):
                    nc.tensor.matmul(pm[0:P, 0:cw],
                                     xT[dti][:, ts_:ts_ + P],
                                     vwT[dti][:, cs:cs + cw],
                                     start=(dti == 0), stop=(dti == DT - 1))
                pmv = pm.rearrange("p (h c) -> p h c", c=HD)
                nc.vector.tensor_copy(vv[0:P, ci * 6:(ci + 1) * 6, 0:HD],
                                      pmv[0:P, 0:6, :])

        def make_state(b):
            state[b] = {
                "xT": [xT_p.tile([128, TPX], BF16, tag=f"xT{dti}", name=f"xT{dti}_{b}") for dti in range(DT)],
                "qkT": [qkT_p.tile([128, TP], BF16, tag=f"qkT{et}", name=f"qkT{et}_{b}") for et in range(12)],
                "v": [v_p.tile([128, H * (HD + 1)], BF16, tag=f"v{ti}", name=f"v{ti}_{b}") for ti in range(len(TT))],
            }

        def stage1_units(b):
            return ([(lambda b=b, et=et: u_mm1a(b, et)) for et in range(12)]
                    + [(lambda b=b, ti=ti: u_mm1b(b, ti)) for ti in range(len(TT))])

        def mm2_ji(b, g, ji):
            st = state[b]
            qt, kt = st["qkT"][g], st["qkT"][6 + g]
            js, JP = TT[ji]
            pss = [ps_s.tile([128, TP], F32, tag=f"s{hp}", name=f"pss_{b}_{g}_{ji}_{hp}")
                   for hp in range(2)]
            for (cs, cw) in CH2:
                for hp in range(2):
                    par = hp * 64
                    nc.tensor.matmul(pss[hp][0:JP, cs:cs + cw],
                                     kt[par:par + 64, js:js + JP],
                                     qt[par:par + 64, cs:cs + cw],
                                     start=True, stop=True)
            for hp in range(2):
                es = es_p.tile([128, TP], BF16, tag=f"es{ji}_{hp}", name=f"es_{b}_{g}_{ji}_{hp}")
                nc.scalar.activation(es[0:JP, 0:TP], pss[hp][0:JP, 0:TP], AF.Exp, scale=SCALE)
                st.setdefault("es", {})[(g, ji, hp)] = es

        def u_mm3(b, g, hp, ci):
            st = state[b]
            h = 2 * g + hp
            par = (h % 2) * 64
            cs, cw = CH2[ci]
            cwv = cw if ci == 0 else cw - 1
            po = ps_o.tile([128, 512], F32, tag="o", name=f"po_{b}_{h}_{ci}")
            for ji in range(len(TT)):
                js, JP = TT[ji]
                es = st["es"][(g, ji, hp)]
                nc.tensor.matmul(po[0:HD + 1, 0:cw],
                                 st["v"][ji][0:JP, h * (HD + 1):(h + 1) * (HD + 1)],
                                 es[0:JP, cs:cs + cw],
                                 start=(ji == 0), stop=(ji == len(TT) - 1))
            nc.vector.tensor_copy(st["oT"][g][par:par + 64, cs:cs + cwv], po[0:HD, 0:cwv])
            dh = st["dh"][h]
            nc.vector.tensor_copy(dh[:, cs:cs + cwv], po[HD:HD + 1, 0:cwv])
            if ci == len(CH2) - 1:
                nc.gpsimd.dma_start(st["rdr_den"][h:h + 1, 0:T], dh[:, 0:T])

        def attn_norm(b):
            st = state[b]
            den = nrm_p.tile([12, TP], F32, tag="den", name=f"den_{b}")
            nc.gpsimd.dma_start(den[:, 0:T], st["rdr_den"][:, 0:T])
            lnd = nrm_p.tile([12, TP], F32, tag="lnd", name=f"lnd_{b}")
            nc.scalar.activation(lnd[:, 0:T], den[:, 0:T], AF.Ln)
            rec = nrm_p.tile([12, TP], BF16, tag="rec", name=f"rec_{b}")
            nc.scalar.activation(rec[:, 0:T], lnd[:, 0:T], AF.Exp, scale=-1.0)
            rdr2 = drp.tile([12, TP], BF16, tag="rdr2", name=f"rdr2_{b}")
            nc.gpsimd.dma_start(rdr2[:, 0:T], rec[:, 0:T])
            for g in range(6):
                bc = bc_p.tile([128, TP], BF16, tag="bc", name=f"bc_{b}_{g}")
                for hp in range(2):
                    par = hp * 64
                    nc.gpsimd.dma_start(bc[par:par + 64, 0:T],
                                        rdr2[2 * g + hp:2 * g + hp + 1, 0:T].to_broadcast((64, T)))
                nc.vector.tensor_tensor(st["oT"][g][:, 0:T], st["oT"][g][:, 0:T],
                                        bc[:, 0:T], mybir.AluOpType.mult)

        def u_mm4(b, ti):
            st = state[b]
            ts_, P = TT[ti]
            ys = yout.tile([128, D], F32, tag="y_sb", name=f"ys_{b}_{ti}")
            for (cs, cw) in ECH:
                pm = ps_mm.tile([128, 512], F32, tag="mm", name=f"pmc_{b}_{ti}_{cs}")
                for dti in range(DT):
                    nc.tensor.matmul(pm[0:P, 0:cw],
                                     st["oT"][dti][:, ts_:ts_ + P],
                                     pwT[dti][:, cs:cs + cw],
                                     start=(dti == 0), stop=(dti == DT - 1))
                nc.vector.tensor_tensor(ys[0:P, cs:cs + cw], pm[0:P, 0:cw],
                                        pb_bc[0:P, cs:cs + cw], mybir.AluOpType.add)
            nc.gpsimd.dma_start(y_d[b * T + ts_: b * T + ts_ + P, :], ys[0:P, :])

        def attn_emit(b, unit_queue):
            st = state[b]
            st["oT"] = [oT_p.tile([128, TP], BF16, tag=f"oT{dti}", name=f"oT{dti}_{b}") for dti in range(DT)]
            st["dh"] = [dh_p.tile([1, TP], F32, tag="dh", name=f"dh_{b}_{h}") for h in range(H)]
            st["rdr_den"] = drp.tile([12, TP], F32, tag="rdr_den", name=f"rdrden_{b}")

            def pull(k):
                for _ in range(k):
                    if unit_queue:
                        unit_queue.pop(0)()

            for g in range(6):
                mm3_units = []
                if g > 0:
                    mm3_units = [(g - 1, hp, ci) for hp in range(2) for ci in range(len(CH2))]
                for ji in range(len(TT)):
                    mm2_ji(b, g, ji)
                    if mm3_units:
                        pg, hp, ci = mm3_units.pop(0)
                        u_mm3(b, pg, hp, ci)
                        pull(1)
                    else:
                        pull(1 if g == 0 else 2)
                pull(1)
            for hp in range(2):
                for ci in range(len(CH2)):
                    u_mm3(b, 5, hp, ci)
                    pull(1)
            attn_norm(b)

        def spread(primary, filler):
            """interleave filler units among primary units roughly evenly"""
            out = []
            if not primary:
                return list(filler)
            step = max(1, len(primary) // max(1, len(filler)))
            fi = 0
            for i, u in enumerate(primary):
                out.append(u)
                if fi < len(filler) and (i + 1) % step == 0:
                    out.append(filler[fi]); fi += 1
            out.extend(filler[fi:])
            return out

        # ---------- startup ----------
        make_state(0)
        for ti in range(len(TT)):
            u_ld(0, ti)

        ones_col_f = wpool.tile([128, 1], F32, tag="ones_col_f")
        nc.gpsimd.memset(ones_col_f[:], 1.0)
        ones_col = wpool.tile([128, 1], BF16, tag="ones_col")
        nc.vector.tensor_copy(ones_col[:], ones_col_f[:])
        qkb_sb = wpool.tile([128, 12], F32, tag="qkb")
        nc.sync.dma_start(qkb_sb[:], qkb_d[:])
        pb_bc = wpool.tile([128, D], F32, tag="pb_bc")
        nc.sync.dma_start(pb_bc[:], pb_d[0:1, :].to_broadcast((128, D)))
        qkwT, vwT, pwT = [], [], []
        for dti in range(DT):
            w = wpool.tile([128, EQK], BF16, tag=f"qkwT{dti}", name=f"qkwT{dti}")
            nc.sync.dma_start(w[:], qkwT_d[dti * 128:(dti + 1) * 128, :])
            qkwT.append(w)
        for dti in range(DT):
            w = wpool.tile([128, D], BF16, tag=f"vwT{dti}", name=f"vwT{dti}")
            nc.sync.dma_start(w[:], vwT_d[dti * 128:(dti + 1) * 128, :])
            vwT.append(w)
        for dti in range(DT):
            w = wpool.tile([128, D], BF16, tag=f"pwT{dti}", name=f"pwT{dti}")
            nc.sync.dma_start(w[:], pwT_d[dti * 128:(dti + 1) * 128, :])
            pwT.append(w)

        # ---------- schedule ----------
        deferred = []
        for u in stage1_units(0):
            u()
        for b in range(nbatch):
            if b + 1 < nbatch:
                make_state(b + 1)
                for ti in range(len(TT)):
                    u_ld(b + 1, ti)
            queue = []
            if b + 1 < nbatch:
                mm4u = [(lambda b=b, ti=ti: u_mm4(b - 1, ti)) for ti in range(len(TT))] if b > 0 else []
                keep, defer = mm4u[:3], mm4u[3:]
                deferred.extend(defer)
                queue = spread(stage1_units(b + 1), keep)
            else:
                queue = deferred + [(lambda b=b, ti=ti: u_mm4(b - 1, ti)) for ti in range(len(TT))]
                deferred = []
            attn_emit(b, queue)
            for u in queue:
                u()
            if b == nbatch - 1:
                for ti in range(len(TT)):
                    u_mm4(b, ti)
    return nc


def host_inputs(x_c, qkv_w, qkv_b, proj_w, proj_b):
    import ml_dtypes
    BF = ml_dtypes.bfloat16
    xp = np.zeros((B, TPAD, D), np.float32)
    xp[:, :T] = np.asarray(x_c, np.float32).reshape(B, T, D)
    qkwT = np.ascontiguousarray(qkv_w[0:EQK].T)
    vwT = np.ascontiguousarray(qkv_w[EQK:2304].T)
    b_v = qkv_b[EQK:2304]
    pb_aug = proj_b + b_v @ proj_w.T
    pwT = np.ascontiguousarray(proj_w.T)
    qkb = np.ascontiguousarray(qkv_b[0:EQK].reshape(12, 128).T)
    return {
        "x": np.ascontiguousarray(xp.reshape(B * TPAD, D)).astype(BF),
        "qkwT": qkwT.astype(BF),
        "vwT": vwT.astype(BF),
        "pwT": pwT.astype(BF),
        "pb": pb_aug[None, :].astype(np.float32),
        "qkb": qkb.astype(np.float32),
    }


import sys as _sys
import numpy as _np

def _split_waits(nc, max_waits=1):
    import concourse.mybir as mybir
    nid = [0]
    for f in nc.m.functions:
        for bb in f.blocks:
            newlist = []; changed = False
            for ins in bb.instructions:
                si = getattr(ins, 'sync_info', None)
                if si is not None and si.on_wait is not None and len(si.on_wait) > max_waits:
                    waits = list(si.on_wait)
                    extra, keep = waits[:-max_waits], waits[-max_waits:]
                    for i in range(0, len(extra), max_waits):
                        nop = mybir.InstNoOp(name=f"I-ws-{nid[0]}", ins=[], outs=[],
                            engine=ins.engine,
                            sync_info=mybir.SyncInfo(on_wait=extra[i:i+max_waits], on_update=[]))
                        nid[0] += 1; newlist.append(nop); changed = True
                    si.on_wait = keep; ins.sync_info = si
                newlist.append(ins)
            if changed:
                bb.instructions = newlist


_NC_CACHE = {}

def _get_nc():
    if "nc" not in _NC_CACHE:
        nc = build(nbatch=B)
        _split_waits(nc)
        _NC_CACHE["nc"] = nc
    return _NC_CACHE["nc"]


def kernel(x, qkv_w, qkv_b, proj_w, proj_b):
    """Full inputs in ([32,577,768] etc.), full output out.

    Data-parallel over batch: 32 batches -> 8 NeuronCores x 4 each. Weights
    replicated (host-side transpose/cast is layout prep only); all compute on
    device (Bass/Tile, bf16 matmuls with fp32 accumulation, software-
    pipelined batch stages)."""
    x = _np.asarray(x, dtype=_np.float32)
    qkv_w = _np.asarray(qkv_w, dtype=_np.float32)
    qkv_b = _np.asarray(qkv_b, dtype=_np.float32)
    proj_w = _np.asarray(proj_w, dtype=_np.float32)
    proj_b = _np.asarray(proj_b, dtype=_np.float32)
    from concourse.bass_utils import run_bass_kernel_spmd
    nc = _get_nc()
    in_maps = [host_inputs(x[c * B:(c + 1) * B], qkv_w, qkv_b, proj_w, proj_b)
               for c in range(8)]
    res = run_bass_kernel_spmd(nc, in_maps, list(range(8)))
    y = _np.concatenate([res.results[c]["y"].reshape(B, T, D) for c in range(8)], axis=0)
    return y.astype(_np.float32)


# revision 7
# speedup vs baseline: 1.2830x; 1.2830x over previous
"""v7: host-prepped bf16 inputs; x transposed on the DMA xbar (PE-free);
MM2 head-pair row-packing; 578-wide merged exp from 2-bank PSUM tiles;
bias-free MM4; exp(-ln(den)) normalization; DMA queue split (sync=x/weights,
gpsimd=den/bc/y); emission-interleaved schedule with deferred-MM4 tail filler."""
import numpy as np
import concourse.bass as bass
import concourse.mybir as mybir
import concourse.tile as tile

dt = mybir.dt
F32 = dt.float32
BF16 = dt.bfloat16
AF = mybir.ActivationFunctionType

B = 4
T = 577
TPAD = 592          # host zero-pads each batch to 592 tokens (16-aligned for xbar)
D = 768
H = 12
HD = 64
EQK = 1536
SCALE = HD ** -0.5

TT = [(i * 128, min(128, T - i * 128)) for i in range((T + 127) // 128)]
TX = [(i * 128, min(128, TPAD - i * 128)) for i in range(5)]   # xbar transpose tiles
TP = 578
TPX = 592
CH2 = [(0, 512), (512, 66)]     # MM2/MM3 q chunks (bank-aligned, merged ACT)
CH1 = [(0, 512), (512, 66)]     # MM1a token chunks
ECH = [(0, 384), (384, 384)]    # MM1b / MM4 feature chunks
DT = 6


def build(nbatch=B):
    nc = bass.Bass()
    x_d = nc.dram_tensor("x", [B, D, TP], BF16, kind="ExternalInput")
    qkwT_d = nc.dram_tensor("qkwT", [D, EQK], BF16, kind="ExternalInput")
    vwT_d = nc.dram_tensor("vwT", [D, D], BF16, kind="ExternalInput")
    pwT_d = nc.dram_tensor("pwT", [D, D], BF16, kind="ExternalInput")
    pb_d = nc.dram_tensor("pb", [1, D], F32, kind="ExternalInput")
    qkb_d = nc.dram_tensor("qkb", [128, 12], F32, kind="ExternalInput")
    y_d = nc.dram_tensor("y", [B * T, D], F32, kind="ExternalOutput")

    from contextlib import ExitStack
    with tile.TileContext(nc) as tc, ExitStack() as ctx:
        wpool = ctx.enter_context(tc.tile_pool(name="wpool", bufs=1))

        xin = ctx.enter_context(tc.tile_pool(name="xin", bufs=2))
        xT_p = ctx.enter_context(tc.tile_pool(name="xT", bufs=2))
        qkT_p = ctx.enter_context(tc.tile_pool(name="qkT", bufs=2))
        v_p = ctx.enter_context(tc.tile_pool(name="v", bufs=2))
        es_p = ctx.enter_context(tc.tile_pool(name="es", bufs=2))
        oT_p = ctx.enter_context(tc.tile_pool(name="oT", bufs=3))
        nrm_p = ctx.enter_context(tc.tile_pool(name="nrm", bufs=2))
        dh_p = ctx.enter_context(tc.tile_pool(name="dh", bufs=4))
        bc_p = ctx.enter_context(tc.tile_pool(name="bc", bufs=3))
        yout = ctx.enter_context(tc.tile_pool(name="yout", bufs=2))
        drp = ctx.enter_context(tc.tile_pool(name="dr", bufs=2, space="DRAM"))

        ps_s = ctx.enter_context(tc.tile_pool(name="ps_s", bufs=1, space="PSUM"))
        ps_o = ctx.enter_context(tc.tile_pool(name="ps_o", bufs=2, space="PSUM"))
        ps_mm = ctx.enter_context(tc.tile_pool(name="ps_mm", bufs=2, space="PSUM"))

        state = {}

        def u_ld(b, dti):
            """x arrives pre-transposed from host: plain row DMA"""
            xT = state[b]["xT"]
            nc.sync.dma_start(xT[dti][:, 0:TP],
                              x_d[b, dti * 128:(dti + 1) * 128, 0:TP])

        def u_mm1a(b, et):
            xT = state[b]["xT"]
            qkT = state[b]["qkT"]
            for (cs, cw) in CH1:
                pm = ps_mm.tile([128, 512], F32, tag="mm", name=f"pma_{b}_{et}_{cs}")
                for dti in range(DT):
                    nc.tensor.matmul(pm[:, 0:cw],
                                     qkwT[dti][:, et * 128:(et + 1) * 128],
                                     xT[dti][:, cs:cs + cw],
                                     start=(dti == 0), stop=(dti == DT - 1))
                nc.vector.tensor_scalar_add(qkT[et][:, cs:cs + cw], pm[:, 0:cw],
                                            qkb_sb[:, et:et + 1])

        def u_mm1b(b, ti):
            xT = state[b]["xT"]
            v_sb = state[b]["v"]
            ts_, P = TT[ti]
            vv = v_sb[ti].rearrange("p (h c) -> p h c", c=HD + 1)
            nc.vector.tensor_copy(vv[0:P, :, HD:HD + 1], ones_col[0:P, :].to_broadcast((P, H, 1)))
            for ci, (cs, cw) in enumerate(ECH):
                pm = ps_mm.tile([128, 512], F32, tag="mm", name=f"pmb_{b}_{ti}_{ci}")
                for dti in range(DT):
                    nc.tensor.matmul(pm[0:P, 0:cw],
                                     xT[dti][:, ts_:ts_ + P],
                                     vwT[dti][:, cs:cs + cw],
                                     start=(dti == 0), stop=(dti == DT - 1))
                pmv = pm.rearrange("p (h c) -> p h c", c=HD)
                nc.vector.tensor_copy(vv[0:P, ci * 6:(ci + 1) * 6, 0:HD],
                                      pmv[0:P, 0:6, :])

        def make_state(b):
            state[b] = {
                "xT": [xT_p.tile([128, TP], BF16, tag=f"xT{dti}", name=f"xT{dti}_{b}") for dti in range(DT)],
                "qkT": [qkT_p.tile([128, TP], BF16, tag=f"qkT{et}", name=f"qkT{et}_{b}") for et in range(12)],
                "v": [v_p.tile([128, H * (HD + 1)], BF16, tag=f"v{ti}", name=f"v{ti}_{b}") for ti in range(len(TT))],
            }

        def stage1_units(b):
            return ([(lambda b=b, et=et: u_mm1a(b, et)) for et in range(12)]
                    + [(lambda b=b, ti=ti: u_mm1b(b, ti)) for ti in range(len(TT))])

        def mm2_ji(b, g, ji):
            st = state[b]
            qt, kt = st["qkT"][g], st["qkT"][6 + g]
            js, JP = TT[ji]
            pss = [ps_s.tile([128, TP], F32, tag=f"s{hp}", name=f"pss_{b}_{g}_{ji}_{hp}")
                   for hp in range(2)]
            for (cs, cw) in CH2:
                for hp in range(2):
                    par = hp * 64
                    nc.tensor.matmul(pss[hp][0:JP, cs:cs + cw],
                                     kt[par:par + 64, js:js + JP],
                                     qt[par:par + 64, cs:cs + cw],
                                     start=True, stop=True)
            for hp in range(2):
                es = es_p.tile([128, TP], BF16, tag=f"es{ji}_{hp}", name=f"es_{b}_{g}_{ji}_{hp}")
                nc.scalar.activation(es[0:JP, 0:TP], pss[hp][0:JP, 0:TP], AF.Exp, scale=SCALE)
                st.setdefault("es", {})[(g, ji, hp)] = es

        def u_mm3(b, g, hp, ci):
            st = state[b]
            h = 2 * g + hp
            par = (h % 2) * 64
            cs, cw = CH2[ci]
            cwv = cw if ci == 0 else cw - 1
            po = ps_o.tile([128, 512], F32, tag="o", name=f"po_{b}_{h}_{ci}")
            for ji in range(len(TT)):
                js, JP = TT[ji]
                es = st["es"][(g, ji, hp)]
                nc.tensor.matmul(po[0:HD + 1, 0:cw],
                                 st["v"][ji][0:JP, h * (HD + 1):(h + 1) * (HD + 1)],
                                 es[0:JP, cs:cs + cw],
                                 start=(ji == 0), stop=(ji == len(TT) - 1))
            nc.vector.tensor_copy(st["oT"][g][par:par + 64, cs:cs + cwv], po[0:HD, 0:cwv])
            dh = st["dh"][h]
            nc.vector.tensor_copy(dh[:, cs:cs + cwv], po[HD:HD + 1, 0:cwv])
            if ci == len(CH2) - 1:
                nc.sync.dma_start(st["rdr_den"][h:h + 1, 0:T], dh[:, 0:T])

        def attn_norm(b):
            st = state[b]
            den = nrm_p.tile([12, TP], F32, tag="den", name=f"den_{b}")
            nc.sync.dma_start(den[:, 0:T], st["rdr_den"][:, 0:T])
            lnd = nrm_p.tile([12, TP], F32, tag="lnd", name=f"lnd_{b}")
            nc.scalar.activation(lnd[:, 0:T], den[:, 0:T], AF.Ln)
            rec = nrm_p.tile([12, TP], BF16, tag="rec", name=f"rec_{b}")
            nc.scalar.activation(rec[:, 0:T], lnd[:, 0:T], AF.Exp, scale=-1.0)
            rdr2 = drp.tile([12, TP], BF16, tag="rdr2", name=f"rdr2_{b}")
            nc.sync.dma_start(rdr2[:, 0:T], rec[:, 0:T])
            for g in range(6):
                bc = bc_p.tile([128, TP], BF16, tag="bc", name=f"bc_{b}_{g}")
                for hp in range(2):
                    par = hp * 64
                    nc.sync.dma_start(bc[par:par + 64, 0:T],
                                      rdr2[2 * g + hp:2 * g + hp + 1, 0:T].to_broadcast((64, T)))
                nc.vector.tensor_tensor(st["oT"][g][:, 0:T], st["oT"][g][:, 0:T],
                                        bc[:, 0:T], mybir.AluOpType.mult)

        def u_mm4(b, ti):
            st = state[b]
            ts_, P = TT[ti]
            ys = yout.tile([128, D], F32, tag="y_sb", name=f"ys_{b}_{ti}")
            for (cs, cw) in ECH:
                pm = ps_mm.tile([128, 512], F32, tag="mm", name=f"pmc_{b}_{ti}_{cs}")
                for dti in range(DT):
                    nc.tensor.matmul(pm[0:P, 0:cw],
                                     st["oT"][dti][:, ts_:ts_ + P],
                                     pwT[dti][:, cs:cs + cw],
                                     start=(dti == 0), stop=(dti == DT - 1))
                nc.vector.tensor_tensor(ys[0:P, cs:cs + cw], pm[0:P, 0:cw],
                                        pb_bc[0:P, cs:cs + cw], mybir.AluOpType.add)
            nc.gpsimd.dma_start(y_d[b * T + ts_: b * T + ts_ + P, :], ys[0:P, :])

        def attn_emit(b, unit_queue):
            st = state[b]
            st["oT"] = [oT_p.tile([128, TP], BF16, tag=f"oT{dti}", name=f"oT{dti}_{b}") for dti in range(DT)]
            st["dh"] = [dh_p.tile([1, TP], F32, tag="dh", name=f"dh_{b}_{h}") for h in range(H)]
            st["rdr_den"] = drp.tile([12, TP], F32, tag="rdr_den", name=f"rdrden_{b}")

            def pull(k):
                for _ in range(k):
                    if unit_queue:
                        unit_queue.pop(0)()

            for g in range(6):
                mm3_units = []
                if g > 0:
                    mm3_units = [(g - 1, hp, ci) for hp in range(2) for ci in range(len(CH2))]
                for ji in range(len(TT)):
                    mm2_ji(b, g, ji)
                    if mm3_units:
                        pg, hp, ci = mm3_units.pop(0)
                        u_mm3(b, pg, hp, ci)
                        pull(1)
                    else:
                        pull(1 if g == 0 else 2)
                pull(1)
            for hp in range(2):
                for ci in range(len(CH2)):
                    u_mm3(b, 5, hp, ci)
                    pull(1)
            attn_norm(b)

        def spread(primary, filler):
            """interleave filler units among primary units roughly evenly"""
            out = []
            if not primary:
                return list(filler)
            step = max(1, len(primary) // max(1, len(filler)))
            fi = 0
            for i, u in enumerate(primary):
                out.append(u)
                if fi < len(filler) and (i + 1) % step == 0:
                    out.append(filler[fi]); fi += 1
            out.extend(filler[fi:])
            return out

        # ---------- startup ----------
        make_state(0)

        qkb_sb = wpool.tile([128, 12], F32, tag="qkb")
        nc.sync.dma_start(qkb_sb[:], qkb_d[:])
        qkwT, vwT, pwT = [], [], []
        for dti in range(DT):
            w = wpool.tile([128, EQK], BF16, tag=f"qkwT{dti}", name=f"qkwT{dti}")
            nc.sync.dma_start(w[:], qkwT_d[dti * 128:(dti + 1) * 128, :])
            qkwT.append(w)
        for dti in range(DT):
            u_ld(0, dti)
        ones_col_f = wpool.tile([128, 1], F32, tag="ones_col_f")
        nc.gpsimd.memset(ones_col_f[:], 1.0)
        ones_col = wpool.tile([128, 1], BF16, tag="ones_col")
        nc.vector.tensor_copy(ones_col[:], ones_col_f[:])
        pb_bc = wpool.tile([128, D], F32, tag="pb_bc")
        nc.sync.dma_start(pb_bc[:], pb_d[0:1, :].to_broadcast((128, D)))
        for dti in range(DT):
            w = wpool.tile([128, D], BF16, tag=f"vwT{dti}", name=f"vwT{dti}")
            nc.sync.dma_start(w[:], vwT_d[dti * 128:(dti + 1) * 128, :])
            vwT.append(w)
        for dti in range(DT):
            w = wpool.tile([128, D], BF16, tag=f"pwT{dti}", name=f"pwT{dti}")
            nc.sync.dma_start(w[:], pwT_d[dti * 128:(dti + 1) * 128, :])
            pwT.append(w)

        # ---------- schedule ----------
        deferred = []
        for u in stage1_units(0):
            u()
        for b in range(nbatch):
            if b + 1 < nbatch:
                make_state(b + 1)
                for dti in range(DT):
                    u_ld(b + 1, dti)
            queue = []
            if b + 1 < nbatch:
                mm4u = [(lambda b=b, ti=ti: u_mm4(b - 1, ti)) for ti in range(len(TT))] if b > 0 else []
                keep, defer = mm4u[:3], mm4u[3:]
                deferred.extend(defer)
                queue = spread(stage1_units(b + 1), keep)
            else:
                queue = deferred + [(lambda b=b, ti=ti: u_mm4(b - 1, ti)) for ti in range(len(TT))]
                deferred = []
            attn_emit(b, queue)
            for u in queue:
                u()
            if b == nbatch - 1:
                for ti in range(len(TT)):
                    u_mm4(b, ti)
    return nc


def host_inputs(x_c, qkv_w, qkv_b, proj_w, proj_b):
    import ml_dtypes
    BF = ml_dtypes.bfloat16
    xp = np.zeros((B, D, TP), np.float32)
    xp[:, :, :T] = np.asarray(x_c, np.float32).reshape(B, T, D).transpose(0, 2, 1)
    qkwT = np.ascontiguousarray(qkv_w[0:EQK].T)
    vwT = np.ascontiguousarray(qkv_w[EQK:2304].T)
    b_v = qkv_b[EQK:2304]
    pb_aug = proj_b + b_v @ proj_w.T
    pwT = np.ascontiguousarray(proj_w.T)
    qkb = np.ascontiguousarray(qkv_b[0:EQK].reshape(12, 128).T)
    return {
        "x": np.ascontiguousarray(xp).astype(BF),
        "qkwT": qkwT.astype(BF),
        "vwT": vwT.astype(BF),
        "pwT": pwT.astype(BF),
        "pb": pb_aug[None, :].astype(np.float32),
        "qkb": qkb.astype(np.float32),
    }


import sys as _sys
import numpy as _np

def _split_waits(nc, max_waits=1):
    import concourse.mybir as mybir
    nid = [0]
    for f in nc.m.functions:
        for bb in f.blocks:
            newlist = []; changed = False
            for ins in bb.instructions:
                si = getattr(ins, 'sync_info', None)
                if si is not None and si.on_wait is not None and len(si.on_wait) > max_waits:
                    waits = list(si.on_wait)
                    extra, keep = waits[:-max_waits], waits[-max_waits:]
                    for i in range(0, len(extra), max_waits):
                        nop = mybir.InstNoOp(name=f"I-ws-{nid[0]}", ins=[], outs=[],
                            engine=ins.engine,
                            sync_info=mybir.SyncInfo(on_wait=extra[i:i+max_waits], on_update=[]))
                        nid[0] += 1; newlist.append(nop); changed = True
                    si.on_wait = keep; ins.sync_info = si
                newlist.append(ins)
            if changed:
                bb.instructions = newlist


_NC_CACHE = {}

def _get_nc():
    if "nc" not in _NC_CACHE:
        nc = build(nbatch=B)
        _split_waits(nc)
        _NC_CACHE["nc"] = nc
    return _NC_CACHE["nc"]


def kernel(x, qkv_w, qkv_b, proj_w, proj_b):
    """Full inputs in ([32,577,768] etc.), full output out.

    Data-parallel over batch: 32 batches -> 8 NeuronCores x 4 each. Weights
    replicated (host-side transpose/cast is layout prep only); all compute on
    device (Bass/Tile, bf16 matmuls with fp32 accumulation, software-
    pipelined batch stages)."""
    x = _np.asarray(x, dtype=_np.float32)
    qkv_w = _np.asarray(qkv_w, dtype=_np.float32)
    qkv_b = _np.asarray(qkv_b, dtype=_np.float32)
    proj_w = _np.asarray(proj_w, dtype=_np.float32)
    proj_b = _np.asarray(proj_b, dtype=_np.float32)
    from concourse.bass_utils import run_bass_kernel_spmd
    nc = _get_nc()
    in_maps = [host_inputs(x[c * B:(c + 1) * B], qkv_w, qkv_b, proj_w, proj_b)
               for c in range(8)]
    res = run_bass_kernel_spmd(nc, in_maps, list(range(8)))
    y = _np.concatenate([res.results[c]["y"].reshape(B, T, D) for c in range(8)], axis=0)
    return y.astype(_np.float32)


# revision 8
# speedup vs baseline: 1.3152x; 1.0251x over previous
"""v7: host-prepped bf16 inputs; x transposed on the DMA xbar (PE-free);
MM2 head-pair row-packing; 578-wide merged exp from 2-bank PSUM tiles;
bias-free MM4; exp(-ln(den)) normalization; DMA queue split (sync=x/weights,
gpsimd=den/bc/y); emission-interleaved schedule with deferred-MM4 tail filler."""
import numpy as np
import concourse.bass as bass
import concourse.mybir as mybir
import concourse.tile as tile

dt = mybir.dt
F32 = dt.float32
BF16 = dt.bfloat16
AF = mybir.ActivationFunctionType

B = 4
T = 577
TPAD = 592          # host zero-pads each batch to 592 tokens (16-aligned for xbar)
D = 768
H = 12
HD = 64
EQK = 1536
SCALE = HD ** -0.5

TT = [(i * 128, min(128, T - i * 128)) for i in range((T + 127) // 128)]
TX = [(i * 128, min(128, TPAD - i * 128)) for i in range(5)]   # xbar transpose tiles
TP = 578
TPX = 592
CH2 = [(0, 512), (512, 66)]     # MM2/MM3 q chunks (bank-aligned, merged ACT)
CH1 = [(0, 512), (512, 66)]     # MM1a token chunks
ECH = [(0, 384), (384, 384)]    # MM1b / MM4 feature chunks
DT = 6


def build(nbatch=B):
    nc = bass.Bass()
    x_d = nc.dram_tensor("x", [B, D, TP], BF16, kind="ExternalInput")
    qkwT_d = nc.dram_tensor("qkwT", [D, EQK], BF16, kind="ExternalInput")
    vwT_d = nc.dram_tensor("vwT", [D, D], BF16, kind="ExternalInput")
    pwT_d = nc.dram_tensor("pwT", [D, D], BF16, kind="ExternalInput")
    pb_d = nc.dram_tensor("pb", [1, D], F32, kind="ExternalInput")
    qkb_d = nc.dram_tensor("qkb", [128, 12], F32, kind="ExternalInput")
    y_d = nc.dram_tensor("y", [B * T, D], F32, kind="ExternalOutput")

    from contextlib import ExitStack
    with tile.TileContext(nc) as tc, ExitStack() as ctx:
        wpool = ctx.enter_context(tc.tile_pool(name="wpool", bufs=1))

        xin = ctx.enter_context(tc.tile_pool(name="xin", bufs=2))
        xT_p = ctx.enter_context(tc.tile_pool(name="xT", bufs=2))
        qkT_p = ctx.enter_context(tc.tile_pool(name="qkT", bufs=2))
        v_p = ctx.enter_context(tc.tile_pool(name="v", bufs=2))
        es_p = ctx.enter_context(tc.tile_pool(name="es", bufs=2))
        oT_p = ctx.enter_context(tc.tile_pool(name="oT", bufs=3))
        nrm_p = ctx.enter_context(tc.tile_pool(name="nrm", bufs=2))
        dh_p = ctx.enter_context(tc.tile_pool(name="dh", bufs=4))
        bc_p = ctx.enter_context(tc.tile_pool(name="bc", bufs=3))
        yout = ctx.enter_context(tc.tile_pool(name="yout", bufs=2))
        drp = ctx.enter_context(tc.tile_pool(name="dr", bufs=2, space="DRAM"))

        ps_s = ctx.enter_context(tc.tile_pool(name="ps_s", bufs=1, space="PSUM"))
        ps_o = ctx.enter_context(tc.tile_pool(name="ps_o", bufs=2, space="PSUM"))
        ps_mm = ctx.enter_context(tc.tile_pool(name="ps_mm", bufs=2, space="PSUM"))

        state = {}

        def u_ld(b, dti):
            """x arrives pre-transposed from host: plain row DMA"""
            xT = state[b]["xT"]
            nc.gpsimd.dma_start(xT[dti][:, 0:TP],
                                x_d[b, dti * 128:(dti + 1) * 128, 0:TP])

        def u_mm1a(b, et):
            xT = state[b]["xT"]
            qkT = state[b]["qkT"]
            for (cs, cw) in CH1:
                pm = ps_mm.tile([128, 512], F32, tag="mm", name=f"pma_{b}_{et}_{cs}")
                for dti in range(DT):
                    nc.tensor.matmul(pm[:, 0:cw],
                                     qkwT[dti][:, et * 128:(et + 1) * 128],
                                     xT[dti][:, cs:cs + cw],
                                     start=(dti == 0), stop=(dti == DT - 1))
                nc.vector.tensor_scalar_add(qkT[et][:, cs:cs + cw], pm[:, 0:cw],
                                            qkb_sb[:, et:et + 1])

        def u_mm1b(b, ti):
            xT = state[b]["xT"]
            v_sb = state[b]["v"]
            ts_, P = TT[ti]
            vv = v_sb[ti].rearrange("p (h c) -> p h c", c=HD + 1)
            nc.vector.tensor_copy(vv[0:P, :, HD:HD + 1], ones_col[0:P, :].to_broadcast((P, H, 1)))
            for ci, (cs, cw) in enumerate(ECH):
                pm = ps_mm.tile([128, 512], F32, tag="mm", name=f"pmb_{b}_{ti}_{ci}")
                for dti in range(DT):
                    nc.tensor.matmul(pm[0:P, 0:cw],
                                     xT[dti][:, ts_:ts_ + P],
                                     vwT[dti][:, cs:cs + cw],
                                     start=(dti == 0), stop=(dti == DT - 1))
                pmv = pm.rearrange("p (h c) -> p h c", c=HD)
                nc.vector.tensor_copy(vv[0:P, ci * 6:(ci + 1) * 6, 0:HD],
                                      pmv[0:P, 0:6, :])

        def make_state(b):
            state[b] = {
                "xT": [xT_p.tile([128, TP], BF16, tag=f"xT{dti}", name=f"xT{dti}_{b}") for dti in range(DT)],
                "qkT": [qkT_p.tile([128, TP], BF16, tag=f"qkT{et}", name=f"qkT{et}_{b}") for et in range(12)],
                "v": [v_p.tile([128, H * (HD + 1)], BF16, tag=f"v{ti}", name=f"v{ti}_{b}") for ti in range(len(TT))],
            }

        def stage1_units(b):
            return ([(lambda b=b, et=et: u_mm1a(b, et)) for et in range(12)]
                    + [(lambda b=b, ti=ti: u_mm1b(b, ti)) for ti in range(len(TT))])

        def mm2_ji(b, g, ji):
            st = state[b]
            qt, kt = st["qkT"][g], st["qkT"][6 + g]
            js, JP = TT[ji]
            pss = [ps_s.tile([128, TP], F32, tag=f"s{hp}", name=f"pss_{b}_{g}_{ji}_{hp}")
                   for hp in range(2)]
            for (cs, cw) in CH2:
                for hp in range(2):
                    par = hp * 64
                    nc.tensor.matmul(pss[hp][0:JP, cs:cs + cw],
                                     kt[par:par + 64, js:js + JP],
                                     qt[par:par + 64, cs:cs + cw],
                                     start=True, stop=True)
            for hp in range(2):
                es = es_p.tile([128, TP], BF16, tag=f"es{ji}_{hp}", name=f"es_{b}_{g}_{ji}_{hp}")
                nc.scalar.activation(es[0:JP, 0:TP], pss[hp][0:JP, 0:TP], AF.Exp, scale=SCALE)
                st.setdefault("es", {})[(g, ji, hp)] = es

        def u_mm3(b, g, hp, ci):
            st = state[b]
            h = 2 * g + hp
            par = (h % 2) * 64
            cs, cw = CH2[ci]
            cwv = cw if ci == 0 else cw - 1
            po = ps_o.tile([128, 512], F32, tag="o", name=f"po_{b}_{h}_{ci}")
            for ji in range(len(TT)):
                js, JP = TT[ji]
                es = st["es"][(g, ji, hp)]
                nc.tensor.matmul(po[0:HD + 1, 0:cw],
                                 st["v"][ji][0:JP, h * (HD + 1):(h + 1) * (HD + 1)],
                                 es[0:JP, cs:cs + cw],
                                 start=(ji == 0), stop=(ji == len(TT) - 1))
            nc.vector.tensor_copy(st["oT"][g][par:par + 64, cs:cs + cwv], po[0:HD, 0:cwv])
            dh = st["dh"][h]
            nc.vector.tensor_copy(dh[:, cs:cs + cwv], po[HD:HD + 1, 0:cwv])
            if ci == len(CH2) - 1:
                nc.sync.dma_start(st["rdr_den"][h:h + 1, 0:T], dh[:, 0:T])

        def attn_norm(b):
            st = state[b]
            den = nrm_p.tile([12, TP], F32, tag="den", name=f"den_{b}")
            nc.sync.dma_start(den[:, 0:T], st["rdr_den"][:, 0:T])
            lnd = nrm_p.tile([12, TP], F32, tag="lnd", name=f"lnd_{b}")
            nc.scalar.activation(lnd[:, 0:T], den[:, 0:T], AF.Ln)
            rec = nrm_p.tile([12, TP], BF16, tag="rec", name=f"rec_{b}")
            nc.scalar.activation(rec[:, 0:T], lnd[:, 0:T], AF.Exp, scale=-1.0)
            rdr2 = drp.tile([12, TP], BF16, tag="rdr2", name=f"rdr2_{b}")
            nc.sync.dma_start(rdr2[:, 0:T], rec[:, 0:T])
            for g in range(6):
                bc = bc_p.tile([128, TP], BF16, tag="bc", name=f"bc_{b}_{g}")
                for hp in range(2):
                    par = hp * 64
                    nc.sync.dma_start(bc[par:par + 64, 0:T],
                                      rdr2[2 * g + hp:2 * g + hp + 1, 0:T].to_broadcast((64, T)))
                nc.vector.tensor_tensor(st["oT"][g][:, 0:T], st["oT"][g][:, 0:T],
                                        bc[:, 0:T], mybir.AluOpType.mult)

        def u_mm4(b, ti):
            st = state[b]
            ts_, P = TT[ti]
            ys = yout.tile([128, D], F32, tag="y_sb", name=f"ys_{b}_{ti}")
            for (cs, cw) in ECH:
                pm = ps_mm.tile([128, 512], F32, tag="mm", name=f"pmc_{b}_{ti}_{cs}")
                for dti in range(DT):
                    nc.tensor.matmul(pm[0:P, 0:cw],
                                     st["oT"][dti][:, ts_:ts_ + P],
                                     pwT[dti][:, cs:cs + cw],
                                     start=(dti == 0), stop=(dti == DT - 1))
                nc.vector.tensor_tensor(ys[0:P, cs:cs + cw], pm[0:P, 0:cw],
                                        pb_bc[0:P, cs:cs + cw], mybir.AluOpType.add)
            nc.sync.dma_start(y_d[b * T + ts_: b * T + ts_ + P, :], ys[0:P, :])

        def attn_emit(b, unit_queue):
            st = state[b]
            st["oT"] = [oT_p.tile([128, TP], BF16, tag=f"oT{dti}", name=f"oT{dti}_{b}") for dti in range(DT)]
            st["dh"] = [dh_p.tile([1, TP], F32, tag="dh", name=f"dh_{b}_{h}") for h in range(H)]
            st["rdr_den"] = drp.tile([12, TP], F32, tag="rdr_den", name=f"rdrden_{b}")

            def pull(k, reserve=4):
                for _ in range(k):
                    if len(unit_queue) > reserve:
                        unit_queue.pop(0)()

            for g in range(6):
                mm3_units = []
                if g > 0:
                    mm3_units = [(g - 1, hp, ci) for hp in range(2) for ci in range(len(CH2))]
                for ji in range(len(TT)):
                    mm2_ji(b, g, ji)
                    if mm3_units:
                        pg, hp, ci = mm3_units.pop(0)
                        u_mm3(b, pg, hp, ci)
                        pull(1)
                    else:
                        pull(1 if g == 0 else 2)
                pull(1)
            for hp in range(2):
                for ci in range(len(CH2)):
                    u_mm3(b, 5, hp, ci)
                    pull(1, reserve=2)
            attn_norm(b)

        def spread(primary, filler):
            """interleave filler units among primary units roughly evenly"""
            out = []
            if not primary:
                return list(filler)
            step = max(1, len(primary) // max(1, len(filler)))
            fi = 0
            for i, u in enumerate(primary):
                out.append(u)
                if fi < len(filler) and (i + 1) % step == 0:
                    out.append(filler[fi]); fi += 1
            out.extend(filler[fi:])
            return out

        # ---------- startup ----------
        make_state(0)

        qkb_sb = wpool.tile([128, 12], F32, tag="qkb")
        nc.sync.dma_start(qkb_sb[:], qkb_d[:])
        qkwT, vwT, pwT = [], [], []
        for dti in range(DT):
            w = wpool.tile([128, EQK], BF16, tag=f"qkwT{dti}", name=f"qkwT{dti}")
            nc.sync.dma_start(w[:], qkwT_d[dti * 128:(dti + 1) * 128, :])
            qkwT.append(w)
        for dti in range(DT):
            u_ld(0, dti)
        ones_col_f = wpool.tile([128, 1], F32, tag="ones_col_f")
        nc.gpsimd.memset(ones_col_f[:], 1.0)
        ones_col = wpool.tile([128, 1], BF16, tag="ones_col")
        nc.vector.tensor_copy(ones_col[:], ones_col_f[:])
        pb_bc = wpool.tile([128, D], F32, tag="pb_bc")
        nc.gpsimd.dma_start(pb_bc[:], pb_d[0:1, :].to_broadcast((128, D)))
        for dti in range(DT):
            w = wpool.tile([128, D], BF16, tag=f"vwT{dti}", name=f"vwT{dti}")
            nc.gpsimd.dma_start(w[:], vwT_d[dti * 128:(dti + 1) * 128, :])
            vwT.append(w)
        for dti in range(DT):
            w = wpool.tile([128, D], BF16, tag=f"pwT{dti}", name=f"pwT{dti}")
            nc.gpsimd.dma_start(w[:], pwT_d[dti * 128:(dti + 1) * 128, :])
            pwT.append(w)

        # ---------- schedule ----------
        deferred = []
        for u in stage1_units(0):
            u()
        for b in range(nbatch):
            if b + 1 < nbatch:
                make_state(b + 1)
                for dti in range(DT):
                    u_ld(b + 1, dti)
            queue = []
            if b + 1 < nbatch:
                mm4u = [(lambda b=b, ti=ti: u_mm4(b - 1, ti)) for ti in range(len(TT))] if b > 0 else []
                keep, defer = mm4u[:3], mm4u[3:]
                deferred.extend(defer)
                queue = spread(stage1_units(b + 1), keep)
            else:
                queue = deferred + [(lambda b=b, ti=ti: u_mm4(b - 1, ti)) for ti in range(len(TT))]
                deferred = []
            attn_emit(b, queue)
            for u in queue:
                u()
            if b == nbatch - 1:
                for ti in range(len(TT)):
                    u_mm4(b, ti)
    return nc


def host_inputs(x_c, qkv_w, qkv_b, proj_w, proj_b):
    import ml_dtypes
    BF = ml_dtypes.bfloat16
    xp = np.zeros((B, D, TP), np.float32)
    xp[:, :, :T] = np.asarray(x_c, np.float32).reshape(B, T, D).transpose(0, 2, 1)
    qkwT = np.ascontiguousarray(qkv_w[0:EQK].T)
    vwT = np.ascontiguousarray(qkv_w[EQK:2304].T)
    b_v = qkv_b[EQK:2304]
    pb_aug = proj_b + b_v @ proj_w.T
    pwT = np.ascontiguousarray(proj_w.T)
    qkb = np.ascontiguousarray(qkv_b[0:EQK].reshape(12, 128).T)
    return {
        "x": np.ascontiguousarray(xp).astype(BF),
        "qkwT": qkwT.astype(BF),
        "vwT": vwT.astype(BF),
        "pwT": pwT.astype(BF),
        "pb": pb_aug[None, :].astype(np.float32),
        "qkb": qkb.astype(np.float32),
    }


import sys as _sys
import numpy as _np

def _split_waits(nc, max_waits=1):
    import concourse.mybir as mybir
    nid = [0]
    for f in nc.m.functions:
        for bb in f.blocks:
            newlist = []; changed = False
            for ins in bb.instructions:
                si = getattr(ins, 'sync_info', None)
                if si is not None and si.on_wait is not None and len(si.on_wait) > max_waits:
                    waits = list(si.on_wait)
                    extra, keep = waits[:-max_waits], waits[-max_waits:]
                    for i in range(0, len(extra), max_waits):
                        nop = mybir.InstNoOp(name=f"I-ws-{nid[0]}", ins=[], outs=[],
                            engine=ins.engine,
                            sync_info=mybir.SyncInfo(on_wait=extra[i:i+max_waits], on_update=[]))
                        nid[0] += 1; newlist.append(nop); changed = True
                    si.on_wait = keep; ins.sync_info = si
                newlist.append(ins)
            if changed:
                bb.instructions = newlist


_NC_CACHE = {}

def _get_nc():
    if "nc" not in _NC_CACHE:
        nc = build(nbatch=B)
        _split_waits(nc)
        _NC_CACHE["nc"] = nc
    return _NC_CACHE["nc"]


def kernel(x, qkv_w, qkv_b, proj_w, proj_b):
    """Full inputs in ([32,577,768] etc.), full output out.

    Data-parallel over batch: 32 batches -> 8 NeuronCores x 4 each. Weights
    replicated (host-side transpose/cast is layout prep only); all compute on
    device (Bass/Tile, bf16 matmuls with fp32 accumulation, software-
    pipelined batch stages)."""
    x = _np.asarray(x, dtype=_np.float32)
    qkv_w = _np.asarray(qkv_w, dtype=_np.float32)
    qkv_b = _np.asarray(qkv_b, dtype=_np.float32)
    proj_w = _np.asarray(proj_w, dtype=_np.float32)
    proj_b = _np.asarray(proj_b, dtype=_np.float32)
    from concourse.bass_utils import run_bass_kernel_spmd
    nc = _get_nc()
    in_maps = [host_inputs(x[c * B:(c + 1) * B], qkv_w, qkv_b, proj_w, proj_b)
               for c in range(8)]
    res = run_bass_kernel_spmd(nc, in_maps, list(range(8)))
    y = _np.concatenate([res.results[c]["y"].reshape(B, T, D) for c in range(8)], axis=0)
    return y.astype(_np.float32)
